# revision 14
# baseline (speedup 1.0000x reference)
"""Trainium2 Bass kernel for nn_ARAttention (sparse banded attention).

Key insight: the output is a softmax over a band of <= 19 positions around
argmax(prev_attention) per batch row (everything else is exactly 0, and the
global logit-max shift cancels in the normalization). So instead of the
dense [N,T,ENC_H]@[ENC_H,ATT_H] matmul (34 GFLOP, 268 MB read), we compute
the argmax on-device, gather a 32-wide window of input_enc rows per batch
row with an indirect DMA, and run the whole attention pipeline only on
those windows. Sharding: pure data-parallel, batch 32 -> 4 rows on each of
8 cores; weights replicated.
"""

import numpy as np

import concourse.bass as bass
import concourse.mybir as mybir

# ---------------------------------------------------------------------------
# Workaround: this container's walrus rejects the TileContext exit drain when
# it carries many semaphore waits ("Too many sync wait commands").  Split the
# final drain's waits across one nop per logical processor.
import concourse.tile as tile
from concourse.vector_clock import VectorClock, ScopedClock


def _patched_drain_and_barrier(self, tick_clock, wait_clock):
    nc = self.nc
    gc = list(tick_clock.global_clock)
    for p, tick in enumerate(gc):
        if tick > 0:
            nop = nc.sync.nop(nofuse=True)
            partial = VectorClock([tick if i == p else 0 for i in range(len(gc))])
            wait_clock.add_sem_waits(nop.ins, ScopedClock({None: partial}))
    nc.sync.drain()
    nc.all_engine_barrier()
    assert self.sems is not None
    popped = nc._tile_sem_poison_stack.pop()
    assert popped is self._sem_poison
    nc.clear_and_free_semaphores(list(self.sems.allocated().values()))
    nc.all_engine_barrier()


tile.TileContext._drain_and_barrier = _patched_drain_and_barrier
# ---------------------------------------------------------------------------

from concourse.masks import make_identity

f32 = mybir.dt.float32
i32 = mybir.dt.int32
u32 = mybir.dt.uint32
AF = mybir.ActivationFunctionType
ALU = mybir.AluOpType

N, T, ENC_H, ATT_H, DEC_H, OUTD, SPK = 32, 4096, 512, 256, 512, 80, 64
ATT_RANGE, KW = 10, 31
NCORES = 8
R = N // NCORES          # batch rows per core
W = 32                   # gathered window width (>= 2*ATT_RANGE-1)
PAD = 15                 # conv left pad
TP = PAD + T + PAD + 2   # padded prev_attention width (4128)
P = 128


def _emit_core(nc, tc, ctx, x):
    """Emit one full per-core computation. x: dict of dram param handles."""
    cp = ctx.enter_context(tc.tile_pool(name="cp", bufs=1))
    wp = ctx.enter_context(tc.tile_pool(name="wp", bufs=1))
    pp = ctx.enter_context(tc.tile_pool(name="pp", bufs=2, space="PSUM"))
    pq = ctx.enter_context(tc.tile_pool(name="pq", bufs=1, space="PSUM"))

    ident = cp.tile([P, P], f32)
    make_identity(nc, ident[:])

    # ---------------- weights / small inputs -> SBUF -------------------
    Wp1a = wp.tile([P, 2 * DEC_H], f32)
    nc.sync.dma_start(Wp1a[:], x["Wp1"][0:128, :])
    Wp1b = wp.tile([16, 2 * DEC_H], f32)
    nc.sync.dma_start(Wp1b[:], x["Wp1"][128:144, :])
    bp1c = wp.tile([P, 8], f32)
    nc.sync.dma_start(bp1c[:], x["bp1c"][:])
    Wp2 = wp.tile([P, 8 * DEC_H], f32)
    nc.sync.dma_start(
        Wp2[:].rearrange("p (q n) -> p q n", n=DEC_H),
        x["Wp2"][:].rearrange("(q p) n -> p q n", p=P),
    )
    bp2c = wp.tile([P, 4], f32)
    nc.sync.dma_start(bp2c[:], x["bp2c"][:])
    Wdec = wp.tile([P, 4 * ATT_H], f32)
    nc.sync.dma_start(
        Wdec[:].rearrange("p (q n) -> p q n", n=ATT_H),
        x["Wdec"][:].rearrange("(q p) n -> p q n", p=P),
    )
    Wenc = wp.tile([P, 4 * ATT_H], f32)
    nc.sync.dma_start(
        Wenc[:].rearrange("p (q n) -> p q n", n=ATT_H),
        x["Wenc"][:].rearrange("(q p) n -> p q n", p=P),
    )
    bencc = wp.tile([P, 2], f32)
    nc.sync.dma_start(bencc[:], x["bencc"][:])
    Wspk = wp.tile([SPK, ATT_H], f32)
    nc.sync.dma_start(Wspk[:], x["Wspk"][:])
    Wspd = wp.tile([1, ATT_H], f32)
    nc.sync.dma_start(Wspd[:], x["Wspd"][:])
    CW = wp.tile([KW, ATT_H], f32)
    nc.sync.dma_start(CW[:], x["CW"][:])
    Wprojc = wp.tile([P, 2], f32)
    nc.sync.dma_start(Wprojc[:], x["Wprojc"][:])
    dsT0 = wp.tile([P, R], f32)
    nc.sync.dma_start(dsT0[:], x["dsT"][0:128, :])
    dsT1 = wp.tile([16, R], f32)
    nc.sync.dma_start(dsT1[:], x["dsT"][128:144, :])
    spkT = wp.tile([SPK, R], f32)
    nc.sync.dma_start(spkT[:], x["spkT"][:])
    spdT = wp.tile([1, R], f32)
    nc.sync.dma_start(spdT[:], x["spdT"][:])
    lens = wp.tile([1, R], f32)
    nc.sync.dma_start(lens[:], x["lens"][:])

    # ---------------- argmax of prev_attention per row ------------------
    # pa128 layout: partition p=(r,q) r=p>>5 q=p&31, free f: t = q*128+f
    pa128 = cp.tile([P, P], f32)
    for r in range(R):
        nc.sync.dma_start(
            pa128[r * 32 : (r + 1) * 32, :],
            x["pa_pad"][r, PAD : PAD + T].rearrange("(q f) -> q f", f=P),
        )
    mx8 = cp.tile([P, 8], f32)
    nc.vector.max(out=mx8[:], in_=pa128[:])
    mi8 = cp.tile([P, 8], u32)
    nc.vector.max_index(out=mi8[:], in_max=mx8[:], in_values=pa128[:])

    # global candidate index per partition: g = (p & 31) * 128 + local_idx
    pv = cp.tile([P, 1], i32)
    nc.gpsimd.iota(pv[:], pattern=[[1, 1]], base=0, channel_multiplier=1)
    gi = cp.tile([P, 1], i32)
    nc.vector.tensor_scalar(
        out=gi[:], in0=pv[:], scalar1=31, scalar2=7,
        op0=ALU.bitwise_and, op1=ALU.logical_shift_left,
    )
    mi0 = cp.tile([P, 1], i32)
    nc.vector.tensor_copy(mi0[:], mi8[:, 0:1])
    nc.vector.tensor_tensor(out=gi[:], in0=gi[:], in1=mi0[:], op=ALU.add)
    gf = cp.tile([P, 1], f32)
    nc.vector.tensor_copy(gf[:], gi[:])

    # transpose candidate values + indices into partition-0 domain
    vTp = pp.tile([1, P], f32, tag="tp")
    nc.tensor.transpose(out=vTp[:], in_=mx8[:, 0:1], identity=ident[:])
    gTp = pp.tile([1, P], f32, tag="tp")
    nc.tensor.transpose(out=gTp[:], in_=gf[:], identity=ident[:])
    vS = cp.tile([1, P], f32)
    nc.vector.tensor_copy(vS[:], vTp[:])
    gS = cp.tile([1, P], f32)
    nc.vector.tensor_copy(gS[:], gTp[:])

    M = cp.tile([1, R], f32)
    nc.vector.reduce_max(
        out=M[:], in_=vS[:].rearrange("p (r q) -> p r q", q=32),
        axis=mybir.AxisListType.X,
    )
    eqm = cp.tile([1, P], f32)
    nc.vector.tensor_tensor(
        out=eqm[:].rearrange("p (r q) -> p r q", q=32),
        in0=vS[:].rearrange("p (r q) -> p r q", q=32),
        in1=M[:].to_broadcast([1, R, 32]),
        op=ALU.is_ge,
    )
    sc = cp.tile([1, P], f32)
    nc.vector.tensor_scalar(
        out=sc[:], in0=gS[:], scalar1=8192.0, scalar2=-1.0,
        op0=ALU.subtract, op1=ALU.mult,
    )
    nc.vector.tensor_tensor(out=sc[:], in0=sc[:], in1=eqm[:], op=ALU.mult)
    Smax = cp.tile([1, R], f32)
    nc.vector.reduce_max(
        out=Smax[:], in_=sc[:].rearrange("p (r q) -> p r q", q=32),
        axis=mybir.AxisListType.X,
    )
    tstar = cp.tile([1, R], f32)
    nc.vector.tensor_scalar(
        out=tstar[:], in0=Smax[:], scalar1=8192.0, scalar2=-1.0,
        op0=ALU.subtract, op1=ALU.mult,
    )

    # ---------------- band boundaries ----------------------------------
    lo = cp.tile([1, R], f32)
    nc.vector.tensor_scalar(
        out=lo[:], in0=tstar[:], scalar1=float(ATT_RANGE - 1), scalar2=0.0,
        op0=ALU.subtract, op1=ALU.max,
    )
    hi = cp.tile([1, R], f32)
    nc.vector.tensor_scalar_add(hi[:], tstar[:], float(ATT_RANGE - 1))
    lenm1 = cp.tile([1, R], f32)
    nc.vector.tensor_scalar_sub(lenm1[:], lens[:], 1.0)
    nc.vector.tensor_tensor(out=hi[:], in0=hi[:], in1=lenm1[:], op=ALU.min)
    s0 = cp.tile([1, R], f32)
    nc.vector.tensor_scalar_min(s0[:], lo[:], float(T - W))
    s0i = cp.tile([1, R], i32)
    nc.vector.tensor_copy(s0i[:], s0[:])

    # window start broadcast along free dim: sB[0, r*32+j] = s0[r]
    sB = cp.tile([1, P], f32)
    nc.vector.tensor_copy(
        sB[:].rearrange("p (r q) -> p r q", q=32),
        s0[:].to_broadcast([1, R, 32]),
    )
    sBTp = pp.tile([P, 1], f32, tag="tp")
    nc.tensor.transpose(out=sBTp[:], in_=sB[:], identity=ident[0:1, 0:1])
    sBT = cp.tile([P, 1], i32)
    nc.vector.tensor_copy(sBT[:], sBTp[:])

    # gather indices: idx[p] = (p>>5)*4096 + (p&31) + s0[p>>5]
    idx = cp.tile([P, 1], i32)
    nc.vector.tensor_scalar(
        out=idx[:], in0=pv[:], scalar1=5, scalar2=12,
        op0=ALU.arith_shift_right, op1=ALU.logical_shift_left,
    )
    jprt = cp.tile([P, 1], i32)
    nc.vector.tensor_scalar(
        out=jprt[:], in0=pv[:], scalar1=31, scalar2=None, op0=ALU.bitwise_and
    )
    nc.vector.tensor_tensor(out=idx[:], in0=idx[:], in1=jprt[:], op=ALU.add)
    nc.vector.tensor_tensor(out=idx[:], in0=idx[:], in1=sBT[:], op=ALU.add)

    # ---------------- gather enc windows & transpose --------------------
    X = wp.tile([P, ENC_H], f32)
    nc.gpsimd.indirect_dma_start(
        out=X[:],
        out_offset=None,
        in_=x["enc"][:],
        in_offset=bass.IndirectOffsetOnAxis(ap=idx[:, 0:1], axis=0),
    )
    XT = wp.tile([P, ENC_H], f32)
    for q in range(4):
        TQ = pp.tile([P, P], f32, tag="tp")
        nc.tensor.transpose(out=TQ[:], in_=X[:, q * P : (q + 1) * P], identity=ident[:])
        nc.vector.tensor_copy(XT[:, q * P : (q + 1) * P], TQ[:])

    # ---------------- conv windows (im2col via dynamic DMA) -------------
    pawT = wp.tile([KW, P], f32)
    svals = []
    for r in range(R):
        sv = nc.values_load(
            s0i[0:1, r : r + 1],
            engines=(mybir.EngineType.Pool,),
            min_val=0,
            max_val=T - W,
            skip_runtime_bounds_check=True,
        )
        svals.append(sv)
        row = x["pa_pad"][r : r + 1, :]
        sl = row[0:1, bass.ds(sv, PAD + W + PAD + 1)]
        win = bass.AP(sl.tensor, sl.offset, [[1, KW], [1, W]])
        nc.gpsimd.dma_start(out=pawT[0:KW, r * W : (r + 1) * W], in_=win)

    # ---------------- attention bias terms (transposed layout) ----------
    # abeT = softsign(enc_window @ Wenc + b_enc).T  -> [256=2x128, 128]
    ss = wp.tile([P, 2 * P], f32)
    for m in range(2):
        PS = pp.tile([P, P], f32, tag="mm")
        for q in range(4):
            nc.tensor.matmul(
                PS[:],
                lhsT=Wenc[:, q * ATT_H + m * P : q * ATT_H + m * P + P],
                rhs=XT[:, q * P : (q + 1) * P],
                start=(q == 0),
                stop=(q == 3),
            )
        nc.scalar.activation(
            ss[:, m * P : (m + 1) * P], PS[:], AF.Identity, bias=bencc[:, m : m + 1]
        )
    ssa = wp.tile([P, 2 * P], f32)
    nc.scalar.activation(ssa[:], ss[:], AF.Abs)
    nc.vector.tensor_scalar_add(ssa[:], ssa[:], 1.0)
    nc.vector.reciprocal(ssa[:], ssa[:])
    nc.vector.tensor_tensor(out=ss[:], in0=ss[:], in1=ssa[:], op=ALU.mult)

    # conv contribution [256, 128]
    PC0 = pp.tile([P, P], f32, tag="pc")
    PC1 = pp.tile([P, P], f32, tag="pc")
    for m, PC in enumerate((PC0, PC1)):
        nc.tensor.matmul(
            PC[:], lhsT=CW[:, m * P : (m + 1) * P], rhs=pawT[:], start=True, stop=True
        )

    # ---------------- prenet (transposed) -------------------------------
    HTr = wp.tile([P, 8 * R], f32)
    for m in range(8):
        PH = pq.tile([P, R], f32, tag="sm")
        nc.tensor.matmul(
            PH[:], lhsT=Wp1a[:, m * P : (m + 1) * P], rhs=dsT0[:],
            start=True, stop=False,
        )
        nc.tensor.matmul(
            PH[:], lhsT=Wp1b[:, m * P : (m + 1) * P], rhs=dsT1[:],
            start=False, stop=True,
        )
        nc.scalar.activation(
            HTr[:, m * R : (m + 1) * R], PH[:], AF.Relu, bias=bp1c[:, m : m + 1]
        )
    opT = wp.tile([P, 4 * R], f32)
    for m2 in range(4):
        PO = pq.tile([P, R], f32, tag="sm")
        for q in range(8):
            nc.tensor.matmul(
                PO[:],
                lhsT=Wp2[:, q * DEC_H + m2 * P : q * DEC_H + m2 * P + P],
                rhs=HTr[:, q * R : (q + 1) * R],
                start=(q == 0),
                stop=(q == 7),
            )
        nc.scalar.activation(
            opT[:, m2 * R : (m2 + 1) * R], PO[:], AF.Relu, bias=bp2c[:, m2 : m2 + 1]
        )

    # v = (out_prenet @ Wdec + speed outer + softsign(spk @ Wspk)).T [256, 4]
    v = wp.tile([P, 2 * R], f32)
    sk = wp.tile([P, 2 * R], f32)
    ska = wp.tile([P, 2 * R], f32)
    for m in range(2):
        PV = pq.tile([P, R], f32, tag="sm")
        for q2 in range(4):
            nc.tensor.matmul(
                PV[:],
                lhsT=Wdec[:, q2 * ATT_H + m * P : q2 * ATT_H + m * P + P],
                rhs=opT[:, q2 * R : (q2 + 1) * R],
                start=(q2 == 0),
                stop=False,
            )
        nc.tensor.matmul(
            PV[:], lhsT=Wspd[0:1, m * P : (m + 1) * P], rhs=spdT[:],
            start=False, stop=True,
        )
        nc.scalar.copy(v[:, m * R : (m + 1) * R], PV[:])
        PK = pq.tile([P, R], f32, tag="sm")
        nc.tensor.matmul(
            PK[:], lhsT=Wspk[:, m * P : (m + 1) * P], rhs=spkT[:],
            start=True, stop=True,
        )
        nc.scalar.copy(sk[:, m * R : (m + 1) * R], PK[:])
    nc.scalar.activation(ska[:], sk[:], AF.Abs)
    nc.vector.tensor_scalar_add(ska[:], ska[:], 1.0)
    nc.vector.reciprocal(ska[:], ska[:])
    nc.vector.tensor_tensor(out=sk[:], in0=sk[:], in1=ska[:], op=ALU.mult)
    nc.vector.tensor_tensor(out=v[:], in0=v[:], in1=sk[:], op=ALU.add)

    # ---------------- combine, tanh, project ----------------------------
    th = wp.tile([P, 2 * P], f32)
    for m, PC in enumerate((PC0, PC1)):
        e = wp.tile([P, P], f32, tag="e")
        nc.vector.tensor_tensor(
            out=e[:], in0=ss[:, m * P : (m + 1) * P], in1=PC[:], op=ALU.add
        )
        nc.vector.tensor_tensor(
            out=e[:].rearrange("p (r j) -> p r j", j=W),
            in0=e[:].rearrange("p (r j) -> p r j", j=W),
            in1=v[:, m * R : (m + 1) * R].unsqueeze(2).to_broadcast([P, R, W]),
            op=ALU.add,
        )
        nc.scalar.activation(th[:, m * P : (m + 1) * P], e[:], AF.Tanh)
    PL = pq.tile([1, P], f32, tag="pl")
    for m in range(2):
        nc.tensor.matmul(
            PL[:], lhsT=Wprojc[:, m : m + 1], rhs=th[:, m * P : (m + 1) * P],
            start=(m == 0), stop=(m == 1),
        )

    # ---------------- banded softmax ------------------------------------
    rmax = cp.tile([1, R], f32)
    nc.vector.reduce_max(
        out=rmax[:], in_=PL[:].rearrange("p (r j) -> p r j", j=W),
        axis=mybir.AxisListType.X,
    )
    tsub = cp.tile([1, P], f32)
    nc.vector.tensor_tensor(
        out=tsub[:].rearrange("p (r j) -> p r j", j=W),
        in0=PL[:].rearrange("p (r j) -> p r j", j=W),
        in1=rmax[:].to_broadcast([1, R, W]),
        op=ALU.subtract,
    )
    pex = cp.tile([1, P], f32)
    nc.scalar.activation(pex[:], tsub[:], AF.Exp)

    ji = cp.tile([1, P], i32)
    nc.gpsimd.iota(
        ji[:].rearrange("p (r j) -> p r j", j=W),
        pattern=[[0, R], [1, W]],
        base=0,
        channel_multiplier=0,
    )
    pos = cp.tile([1, P], f32)
    nc.vector.tensor_copy(pos[:], ji[:])
    nc.vector.tensor_tensor(out=pos[:], in0=pos[:], in1=sB[:], op=ALU.add)
    m1 = cp.tile([1, P], f32)
    nc.vector.tensor_tensor(
        out=m1[:].rearrange("p (r j) -> p r j", j=W),
        in0=pos[:].rearrange("p (r j) -> p r j", j=W),
        in1=lo[:].to_broadcast([1, R, W]),
        op=ALU.is_ge,
    )
    m2t = cp.tile([1, P], f32)
    nc.vector.tensor_tensor(
        out=m2t[:].rearrange("p (r j) -> p r j", j=W),
        in0=pos[:].rearrange("p (r j) -> p r j", j=W),
        in1=hi[:].to_broadcast([1, R, W]),
        op=ALU.is_le,
    )
    nc.vector.tensor_tensor(out=m1[:], in0=m1[:], in1=m2t[:], op=ALU.mult)
    num = cp.tile([1, P], f32)
    nc.vector.tensor_tensor(out=num[:], in0=pex[:], in1=m1[:], op=ALU.mult)
    den = cp.tile([1, R], f32)
    nc.vector.reduce_sum(
        out=den[:], in_=num[:].rearrange("p (r j) -> p r j", j=W),
        axis=mybir.AxisListType.X,
    )
    nc.vector.tensor_scalar_max(den[:], den[:], 1e-12)
    rden = cp.tile([1, R], f32)
    nc.vector.reciprocal(rden[:], den[:])
    vals = cp.tile([1, P], f32)
    nc.vector.tensor_tensor(
        out=vals[:].rearrange("p (r j) -> p r j", j=W),
        in0=num[:].rearrange("p (r j) -> p r j", j=W),
        in1=rden[:].to_broadcast([1, R, W]),
        op=ALU.mult,
    )

    # ---------------- output: zero + scatter ----------------------------
    Z = cp.tile([P, P], f32)
    nc.vector.memset(Z[:], 0.0)
    out_flat = x["out"][:].rearrange("a b -> (a b)").rearrange("(p f) -> p f", f=P)
    dsem = nc.alloc_semaphore(f"outsem{nc.next_id()}")
    with tc.tile_critical():
        nc.gpsimd.dma_start(out=out_flat, in_=Z[:]).then_inc(dsem, 16)
        for r in range(R):
            nc.gpsimd.dma_start(
                out=x["out"][r : r + 1, bass.ds(svals[r], W)],
                in_=vals[0:1, r * W : (r + 1) * W],
            )._wait_ge(dsem, 16).then_inc(dsem, 16)
    for sv in svals:
        for h in sv.val.handles:
            nc.free_register(h)


def _split_sync_waits(nc, cap: int = 1):
    """This walrus build rejects instructions carrying several semaphore
    waits.  Engines execute their stream in order, so hoisting excess waits
    onto NoOps inserted immediately before the instruction is equivalent."""
    f = nc.m.functions[0]
    uid = [0]
    for blk in f.blocks:
        insts = blk.instructions
        out = []
        for inst in insts:
            si = inst.sync_info
            waits = list(si.on_wait) if (si is not None and si.on_wait) else []
            if len(waits) > cap:
                keep, excess = waits[:cap], waits[cap:]
                for k in range(0, len(excess), cap):
                    nop = mybir.InstEventSemaphore(
                        name=f"{inst.name}-ws{uid[0]}",
                        engine=inst.engine,
                        ins=[],
                        outs=[],
                        sync_info=mybir.SyncInfo(
                            on_wait=excess[k : k + cap], on_update=[]
                        ),
                    )
                    uid[0] += 1
                    out.append(nop)
                inst.sync_info = mybir.SyncInfo(
                    on_wait=keep, on_update=list(si.on_update or [])
                )
            out.append(inst)
        blk.instructions = out


def build_graph(reps: int = 1, split_waits: bool = True):
    from contextlib import ExitStack

    nc = bass.Bass()
    x = {}
    x["pa_pad"] = nc.declare_dram_parameter("pa_pad", [R, TP], f32, isOutput=False)
    x["enc"] = nc.declare_dram_parameter("enc", [R * T, ENC_H], f32, isOutput=False)
    x["dsT"] = nc.declare_dram_parameter("dsT", [OUTD + SPK, R], f32, isOutput=False)
    x["spkT"] = nc.declare_dram_parameter("spkT", [SPK, R], f32, isOutput=False)
    x["spdT"] = nc.declare_dram_parameter("spdT", [1, R], f32, isOutput=False)
    x["lens"] = nc.declare_dram_parameter("lens", [1, R], f32, isOutput=False)
    x["Wp1"] = nc.declare_dram_parameter("Wp1", [OUTD + SPK, 2 * DEC_H], f32, isOutput=False)
    x["bp1c"] = nc.declare_dram_parameter("bp1c", [P, 8], f32, isOutput=False)
    x["Wp2"] = nc.declare_dram_parameter("Wp2", [2 * DEC_H, DEC_H], f32, isOutput=False)
    x["bp2c"] = nc.declare_dram_parameter("bp2c", [P, 4], f32, isOutput=False)
    x["Wdec"] = nc.declare_dram_parameter("Wdec", [DEC_H, ATT_H], f32, isOutput=False)
    x["Wenc"] = nc.declare_dram_parameter("Wenc", [ENC_H, ATT_H], f32, isOutput=False)
    x["bencc"] = nc.declare_dram_parameter("bencc", [P, 2], f32, isOutput=False)
    x["Wspk"] = nc.declare_dram_parameter("Wspk", [SPK, ATT_H], f32, isOutput=False)
    x["Wspd"] = nc.declare_dram_parameter("Wspd", [1, ATT_H], f32, isOutput=False)
    x["CW"] = nc.declare_dram_parameter("CW", [KW, ATT_H], f32, isOutput=False)
    x["Wprojc"] = nc.declare_dram_parameter("Wprojc", [P, 2], f32, isOutput=False)
    x["out"] = nc.declare_dram_parameter("out", [R, T], f32, isOutput=True)

    with tile.TileContext(nc) as tc:
        for _ in range(reps):
            with ExitStack() as ctx:
                _emit_core(nc, tc, ctx, x)
    if split_waits:
        _split_sync_waits(nc)
    return nc


def host_prep(inputs: dict) -> list:
    """Shard + lay out the full inputs into 8 per-core input maps."""
    inp = {k: np.asarray(v) for k, v in inputs.items()}
    pa = inp["prev_attention"].astype(np.float32)[:, :, 0]        # [N, T]
    enc = inp["input_enc"].astype(np.float32)                      # [N, T, E]
    dec = inp["input_dec"].astype(np.float32)[:, 0, :]             # [N, 80]
    spk = inp["spkr_vec"].astype(np.float32)[:, 0, :]              # [N, 64]
    spd = inp["speed"].astype(np.float32)                          # [N]
    lens = inp["lengths_enc"].astype(np.float32)                   # [N]

    def c(a):
        return np.ascontiguousarray(a, dtype=np.float32)

    shared = {
        "Wp1": c(inp["Wp1"]),
        "bp1c": c(inp["bp1"].reshape(8, P).T),
        "Wp2": c(inp["Wp2"]),
        "bp2c": c(inp["bp2"].reshape(4, P).T),
        "Wdec": c(inp["W_dec"]),
        "Wenc": c(inp["W_enc"]),
        "bencc": c(inp["b_enc"].reshape(2, P).T),
        "Wspk": c(inp["W_spkr"]),
        "Wspd": c(inp["W_speed"].reshape(1, ATT_H)),
        "CW": c(inp["conv_w"][:, 0, :].T),
        "Wprojc": c(inp["W_proj"].reshape(2, P).T),
    }
    in_maps = []
    for cix in range(NCORES):
        rows = slice(cix * R, (cix + 1) * R)
        pa_pad = np.zeros((R, TP), np.float32)
        pa_pad[:, PAD : PAD + T] = pa[rows]
        m = {
            "pa_pad": pa_pad,
            "enc": c(enc[rows].reshape(R * T, ENC_H)),
            "dsT": c(np.concatenate([dec[rows], spk[rows]], axis=1).T),
            "spkT": c(spk[rows].T),
            "spdT": c(spd[rows].reshape(1, R)),
            "lens": c(lens[rows].reshape(1, R)),
        }
        m.update(shared)
        in_maps.append(m)
    return in_maps


_CACHED = {}


def kernel(**inputs) -> np.ndarray:
    from concourse.bass_utils import run_bass_kernel_spmd

    if "nc" not in _CACHED:
        _CACHED["nc"] = build_graph()
    nc = _CACHED["nc"]
    in_maps = host_prep(inputs)
    res = run_bass_kernel_spmd(nc, in_maps, core_ids=list(range(NCORES)))
    out = np.empty((N, T, 1), np.float32)
    for cix in range(NCORES):
        out[cix * R : (cix + 1) * R, :, 0] = res.results[cix]["out"]
    return out


# revision 25
# speedup vs baseline: 18125.6039x; 18125.6039x over previous
"""Trainium2 Bass kernel for nn_ARAttention (sparse banded attention).

Key insight: the output is a softmax over a band of <= 19 positions around
argmax(prev_attention) per batch row (everything else is exactly 0, and the
global logit-max shift cancels in the normalization). So instead of the
dense [N,T,ENC_H]@[ENC_H,ATT_H] matmul (34 GFLOP, 268 MB read), we compute
the argmax on-device, gather a 32-wide window of input_enc rows per batch
row with an indirect DMA, and run the whole attention pipeline only on
those windows. Sharding: pure data-parallel, batch 32 -> 4 rows on each of
8 cores; weights replicated.

Data movement is packed to minimize DMA trigger overhead: all small
weights/vectors ship as one [128, C] image, the big weights as four [128,
C] images (host pre-folded into the SBUF layout the matmuls want), with
prev_attention + images on the ACT HWDGE ring and weights on the SP ring.
"""

import numpy as np

import concourse.bass as bass
import concourse.mybir as mybir

# ---------------------------------------------------------------------------
# Workaround: this container's walrus rejects instructions carrying several
# semaphore waits ("Too many sync wait commands").  (1) the TileContext exit
# drain gets its waits split across one nop per logical processor; (2) a
# post-pass hoists excess waits from any instruction onto InstEventSemaphore
# carriers inserted immediately before it (engines run their stream in
# order, so this is equivalent).
import concourse.tile as tile
from concourse.vector_clock import VectorClock, ScopedClock


def _patched_drain_and_barrier(self, tick_clock, wait_clock):
    nc = self.nc
    gc = list(tick_clock.global_clock)
    for p, tick in enumerate(gc):
        if tick > 0:
            nop = nc.sync.nop(nofuse=True)
            partial = VectorClock([tick if i == p else 0 for i in range(len(gc))])
            wait_clock.add_sem_waits(nop.ins, ScopedClock({None: partial}))
    nc.sync.drain()
    nc.all_engine_barrier()
    assert self.sems is not None
    popped = nc._tile_sem_poison_stack.pop()
    assert popped is self._sem_poison
    nc.clear_and_free_semaphores(list(self.sems.allocated().values()))
    nc.all_engine_barrier()


tile.TileContext._drain_and_barrier = _patched_drain_and_barrier
# ---------------------------------------------------------------------------

from concourse.masks import make_identity

f32 = mybir.dt.float32
bf16 = mybir.dt.bfloat16
i32 = mybir.dt.int32
u32 = mybir.dt.uint32
AF = mybir.ActivationFunctionType
ALU = mybir.AluOpType
ENG_ACT = mybir.EngineType.Activation

N, T, ENC_H, ATT_H, DEC_H, OUTD, SPK = 32, 4096, 512, 256, 512, 80, 64
ATT_RANGE, KW = 10, 31
NCORES = 8
R = N // NCORES          # batch rows per core
W = 32                   # gathered window width (>= 2*ATT_RANGE-1)
PAD = 15                 # conv left pad
TP = PAD + T + PAD + 2   # padded prev_attention width (4128)
P = 128

# img_small (f32) column layout
C_BP1, C_BP2, C_BENC, C_WPROJ, C_LEN, C_CW = 0, 8, 12, 14, 16, 20
SMC = 276
# img_small_bf16 column layout
B_DST0, B_DST1, B_SPK, B_SPD, B_WSPK, B_WSPD = 0, 4, 8, 12, 16, 272
B_BENC = 528
SMB = 784


def _emit_core(nc, tc, ctx, x):
    """Emit one full per-core computation. x: dict of dram param handles."""
    cp = ctx.enter_context(tc.tile_pool(name="cp", bufs=1))
    wp = ctx.enter_context(tc.tile_pool(name="wp", bufs=1))
    pp = ctx.enter_context(tc.tile_pool(name="pp", bufs=2, space="PSUM"))
    pq = ctx.enter_context(tc.tile_pool(name="pq", bufs=2, space="PSUM"))
    pr = ctx.enter_context(tc.tile_pool(name="pr", bufs=1, space="PSUM"))

    ident = cp.tile([P, P], f32)
    make_identity(nc, ident[:])

    # ---- input DMAs: ACT ring = pa + small image; SP ring = big weights ----
    pa128 = cp.tile([P, P], f32)
    with tc.high_priority():
        nc.scalar.dma_start(
            pa128[:],
            x["pa"][:].rearrange("r t -> (r t)").rearrange("(a b) -> a b", b=P),
        )
    sm = wp.tile([P, SMC], f32)
    nc.scalar.dma_start(sm[:], x["smimg"][:])
    smb = wp.tile([P, SMB], bf16)
    nc.scalar.dma_start(smb[:], x["smimgb"][:])

    wenc = wp.tile([P, 4 * ATT_H], bf16)
    nc.sync.dma_start(wenc[:], x["wenc"][:])
    wp1 = wp.tile([P, 2 * DEC_H], bf16)
    nc.sync.dma_start(wp1[:], x["wp1"][:])
    wp1b = wp.tile([16, 2 * DEC_H], bf16)
    nc.scalar.dma_start(wp1b[:], x["wp1b"][:])
    wdec = wp.tile([P, 4 * ATT_H], bf16)
    nc.sync.dma_start(wdec[:], x["wdec"][:])
    wp2 = wp.tile([P, 8 * DEC_H], bf16)
    for h in range(2):
        nc.sync.dma_start(
            wp2[:, h * 4 * DEC_H : (h + 1) * 4 * DEC_H],
            x["wp2"][:, h * 4 * DEC_H : (h + 1) * 4 * DEC_H],
        )

    # ---- zero the output early (scatter at the end overwrites the band) ----
    Z = cp.tile([P, P], f32)
    nc.vector.memset(Z[:], 0.0)
    out_flat = x["out"][:].rearrange("a b -> (a b)").rearrange("(p f) -> p f", f=P)
    zero_dma = nc.sync.dma_start(out=out_flat, in_=Z[:])

    # ---------------- argmax of prev_attention per row ------------------
    # pa128 layout: partition p, free f: row r=p>>5, t=(p&31)*128+f
    mx8 = cp.tile([P, 8], f32)
    nc.vector.max(out=mx8[:], in_=pa128[:])
    mi8 = cp.tile([P, 8], u32)
    nc.vector.max_index(out=mi8[:], in_max=mx8[:], in_values=pa128[:])

    # global candidate index per partition: g = (p & 31) * 128 + local_idx
    pv = cp.tile([P, 1], i32)
    nc.gpsimd.iota(pv[:], pattern=[[1, 1]], base=0, channel_multiplier=1)
    gi = cp.tile([P, 1], i32)
    nc.vector.tensor_scalar(
        out=gi[:], in0=pv[:], scalar1=31, scalar2=7,
        op0=ALU.bitwise_and, op1=ALU.logical_shift_left,
    )
    mi0 = cp.tile([P, 1], i32)
    nc.vector.tensor_copy(mi0[:], mi8[:, 0:1])
    nc.vector.tensor_tensor(out=gi[:], in0=gi[:], in1=mi0[:], op=ALU.add)
    gf = cp.tile([P, 1], f32)
    nc.vector.tensor_copy(gf[:], gi[:])

    # transpose candidate values + indices into partition-0 domain
    vTp = pp.tile([1, P], f32, tag="tp")
    nc.tensor.transpose(out=vTp[:], in_=mx8[:, 0:1], identity=ident[:])
    gTp = pp.tile([1, P], f32, tag="tp")
    nc.tensor.transpose(out=gTp[:], in_=gf[:], identity=ident[:])
    vS = cp.tile([1, P], f32)
    nc.vector.tensor_copy(vS[:], vTp[:])
    gSt = cp.tile([1, P], f32)
    nc.vector.tensor_copy(gSt[:], gTp[:])
    gS = gSt[:]
    vS = vS[:]

    M = cp.tile([1, R], f32)
    nc.vector.reduce_max(
        out=M[:], in_=vS.rearrange("p (r q) -> p r q", q=32),
        axis=mybir.AxisListType.X,
    )
    eqm = cp.tile([1, P], f32)
    nc.vector.tensor_tensor(
        out=eqm[:].rearrange("p (r q) -> p r q", q=32),
        in0=vS.rearrange("p (r q) -> p r q", q=32),
        in1=M[:].to_broadcast([1, R, 32]),
        op=ALU.is_ge,
    )
    sc = cp.tile([1, P], f32)
    nc.vector.tensor_scalar(
        out=sc[:], in0=gS, scalar1=8192.0, scalar2=-1.0,
        op0=ALU.subtract, op1=ALU.mult,
    )
    nc.vector.tensor_tensor(out=sc[:], in0=sc[:], in1=eqm[:], op=ALU.mult)
    Smax = cp.tile([1, R], f32)
    nc.vector.reduce_max(
        out=Smax[:], in_=sc[:].rearrange("p (r q) -> p r q", q=32),
        axis=mybir.AxisListType.X,
    )
    tstar = cp.tile([1, R], f32)
    nc.vector.tensor_scalar(
        out=tstar[:], in0=Smax[:], scalar1=8192.0, scalar2=-1.0,
        op0=ALU.subtract, op1=ALU.mult,
    )

    # ---------------- band boundaries ----------------------------------
    lo = cp.tile([1, R], f32)
    nc.vector.tensor_scalar(
        out=lo[:], in0=tstar[:], scalar1=float(ATT_RANGE - 1), scalar2=0.0,
        op0=ALU.subtract, op1=ALU.max,
    )
    hi = cp.tile([1, R], f32)
    nc.vector.tensor_scalar_add(hi[:], tstar[:], float(ATT_RANGE - 1))
    lenm1 = cp.tile([1, R], f32)
    nc.vector.tensor_scalar_sub(lenm1[:], sm[0:1, C_LEN : C_LEN + R], 1.0)
    nc.vector.tensor_tensor(out=hi[:], in0=hi[:], in1=lenm1[:], op=ALU.min)
    s0 = cp.tile([1, R], f32)
    nc.vector.tensor_scalar_min(s0[:], lo[:], float(T - W))
    s0i = cp.tile([1, R], i32)
    nc.vector.tensor_copy(s0i[:], s0[:])

    # window start broadcast along free dim: sB[0, r*32+j] = s0[r]
    sB = cp.tile([1, P], f32)
    nc.vector.tensor_copy(
        sB[:].rearrange("p (r q) -> p r q", q=32),
        s0[:].to_broadcast([1, R, 32]),
    )
    sBTp = pp.tile([P, 1], f32, tag="tp")
    nc.tensor.transpose(out=sBTp[:], in_=sB[:], identity=ident[0:1, 0:1])
    sBT = cp.tile([P, 1], i32)
    nc.vector.tensor_copy(sBT[:], sBTp[:])

    # gather/scatter indices: idx[p] = (p>>5)*4096 + (p&31) + s0[p>>5]
    idx = cp.tile([P, 1], i32)
    nc.vector.tensor_scalar(
        out=idx[:], in0=pv[:], scalar1=5, scalar2=12,
        op0=ALU.arith_shift_right, op1=ALU.logical_shift_left,
    )
    jprt = cp.tile([P, 1], i32)
    nc.vector.tensor_scalar(
        out=jprt[:], in0=pv[:], scalar1=31, scalar2=None, op0=ALU.bitwise_and
    )
    nc.vector.tensor_tensor(out=idx[:], in0=idx[:], in1=jprt[:], op=ALU.add)
    nc.vector.tensor_tensor(out=idx[:], in0=idx[:], in1=sBT[:], op=ALU.add)

    # ---------------- gather enc windows & transpose --------------------
    X = wp.tile([P, ENC_H], bf16)
    nc.gpsimd.indirect_dma_start(
        out=X[:],
        out_offset=None,
        in_=x["enc"][:],
        in_offset=bass.IndirectOffsetOnAxis(ap=idx[:, 0:1], axis=0),
    )
    XT = wp.tile([P, ENC_H], bf16)
    identb = cp.tile([P, P], bf16)
    nc.vector.tensor_copy(identb[:], ident[:])
    for q in range(4):
        TQ = pp.tile([P, P], bf16, tag="tpb")
        nc.tensor.transpose(
            out=TQ[:], in_=X[:, q * P : (q + 1) * P], identity=identb[:]
        )
        nc.vector.tensor_copy(XT[:, q * P : (q + 1) * P], TQ[:])

    # ---------------- conv windows (im2col via dynamic DMA, ACT ring) ----
    pawT = wp.tile([KW, P], f32)
    svals = []
    for r in range(R):
        sv = nc.values_load(
            s0i[0:1, r : r + 1],
            engines=(ENG_ACT, mybir.EngineType.SP, mybir.EngineType.Pool),
            min_val=0,
            max_val=T - W,
            skip_runtime_bounds_check=True,
        )
        svals.append(sv)
        row = x["pa_pad"][r : r + 1, :]
        sl = row[0:1, bass.ds(sv, PAD + W + PAD + 1)]
        win = bass.AP(sl.tensor, sl.offset, [[1, KW], [1, W]])
        eng = nc.scalar if r % 2 == 0 else nc.sync
        eng.dma_start(out=pawT[0:KW, r * W : (r + 1) * W], in_=win)

    # ---------------- attention bias terms (transposed layout) ----------
    # abeT = softsign(enc_window @ Wenc + b_enc).T  -> [256=2x128, 128]
    ss = wp.tile([P, 2 * P], f32)
    ones1 = cp.tile([1, P], bf16)
    nc.vector.memset(ones1[:], 1.0)
    PS = pr.tile([P, 2 * P], f32, tag="mm")
    for m in range(2):
        for q in range(4):
            nc.tensor.matmul(
                PS[:, m * P : (m + 1) * P],
                lhsT=wenc[:, q * ATT_H + m * P : q * ATT_H + m * P + P],
                rhs=XT[:, q * P : (q + 1) * P],
                start=(q == 0),
                stop=False,
            )
        nc.tensor.matmul(
            PS[:, m * P : (m + 1) * P],
            lhsT=smb[0:1, B_BENC + m * P : B_BENC + (m + 1) * P],
            rhs=ones1[0:1, :],
            start=False,
            stop=True,
        )
    ssa = wp.tile([P, 2 * P], f32)
    nc.scalar.activation(ssa[:], PS[:], AF.Abs)
    nc.vector.tensor_scalar_add(ssa[:], ssa[:], 1.0)
    nc.vector.reciprocal(ssa[:], ssa[:])
    nc.vector.tensor_tensor(out=ss[:], in0=PS[:], in1=ssa[:], op=ALU.mult)

    # conv contribution [256, 128]
    PC = pr.tile([P, 2 * P], f32, tag="pc")
    for m in range(2):
        nc.tensor.matmul(
            PC[:, m * P : (m + 1) * P],
            lhsT=sm[0:KW, C_CW + m * P : C_CW + (m + 1) * P],
            rhs=pawT[:],
            start=True,
            stop=True,
        )

    # ---------------- prenet (transposed) -------------------------------
    HTr = wp.tile([P, 8 * R], f32)
    PH = pq.tile([P, 8 * R], f32, tag="sm")
    for m in range(8):
        nc.tensor.matmul(
            PH[:, m * R : (m + 1) * R], lhsT=wp1[:, m * P : (m + 1) * P],
            rhs=smb[:, B_DST0 : B_DST0 + R],
            start=True, stop=False,
        )
        nc.tensor.matmul(
            PH[:, m * R : (m + 1) * R],
            lhsT=wp1b[0:16, m * P : (m + 1) * P],
            rhs=smb[0:16, B_DST1 : B_DST1 + R],
            start=False, stop=True,
        )
    nc.vector.tensor_tensor(
        out=HTr[:].rearrange("p (m r) -> p m r", r=R),
        in0=PH[:].rearrange("p (m r) -> p m r", r=R),
        in1=sm[:, C_BP1 : C_BP1 + 8].unsqueeze(2).to_broadcast([P, 8, R]),
        op=ALU.add,
    )
    nc.vector.tensor_scalar_max(HTr[:], HTr[:], 0.0)
    HTrB = wp.tile([P, 8 * R], bf16)
    nc.vector.tensor_copy(HTrB[:], HTr[:])
    opT = wp.tile([P, 4 * R], f32)
    PO = pq.tile([P, 4 * R], f32, tag="sm")
    for m2 in range(4):
        for q in range(8):
            nc.tensor.matmul(
                PO[:, m2 * R : (m2 + 1) * R],
                lhsT=wp2[:, q * DEC_H + m2 * P : q * DEC_H + m2 * P + P],
                rhs=HTrB[:, q * R : (q + 1) * R],
                start=(q == 0),
                stop=(q == 7),
            )
    nc.vector.tensor_tensor(
        out=opT[:].rearrange("p (m r) -> p m r", r=R),
        in0=PO[:].rearrange("p (m r) -> p m r", r=R),
        in1=sm[:, C_BP2 : C_BP2 + 4].unsqueeze(2).to_broadcast([P, 4, R]),
        op=ALU.add,
    )
    nc.vector.tensor_scalar_max(opT[:], opT[:], 0.0)
    opTB = wp.tile([P, 4 * R], bf16)
    nc.vector.tensor_copy(opTB[:], opT[:])

    # v = (out_prenet @ Wdec + speed outer + softsign(spk @ Wspk)).T [256, 4]
    v = wp.tile([P, 2 * R], f32)
    sk = wp.tile([P, 2 * R], f32)
    ska = wp.tile([P, 2 * R], f32)
    PV = pq.tile([P, 2 * R], f32, tag="sm")
    PK = pq.tile([P, 2 * R], f32, tag="sm")
    for m in range(2):
        for q2 in range(4):
            nc.tensor.matmul(
                PV[:, m * R : (m + 1) * R],
                lhsT=wdec[:, q2 * ATT_H + m * P : q2 * ATT_H + m * P + P],
                rhs=opTB[:, q2 * R : (q2 + 1) * R],
                start=(q2 == 0),
                stop=False,
            )
        nc.tensor.matmul(
            PV[:, m * R : (m + 1) * R],
            lhsT=smb[0:1, B_WSPD + m * P : B_WSPD + (m + 1) * P],
            rhs=smb[0:1, B_SPD : B_SPD + R],
            start=False, stop=True,
        )
        nc.tensor.matmul(
            PK[:, m * R : (m + 1) * R],
            lhsT=smb[0:SPK, B_WSPK + m * P : B_WSPK + (m + 1) * P],
            rhs=smb[0:SPK, B_SPK : B_SPK + R],
            start=True, stop=True,
        )
    nc.scalar.activation(ska[:], PK[:], AF.Abs)
    nc.vector.tensor_scalar_add(ska[:], ska[:], 1.0)
    nc.vector.reciprocal(ska[:], ska[:])
    nc.vector.tensor_tensor(out=sk[:], in0=PK[:], in1=ska[:], op=ALU.mult)
    nc.vector.tensor_tensor(out=v[:], in0=PV[:], in1=sk[:], op=ALU.add)

    # ---------------- combine, tanh, project ----------------------------
    th = wp.tile([P, 2 * P], f32)
    e = wp.tile([P, 2 * P], f32)
    nc.vector.tensor_tensor(out=e[:], in0=ss[:], in1=PC[:], op=ALU.add)
    nc.vector.tensor_tensor(
        out=e[:].rearrange("p (m r j) -> p (m r) j", j=W, m=2),
        in0=e[:].rearrange("p (m r j) -> p (m r) j", j=W, m=2),
        in1=v[:].unsqueeze(2).to_broadcast([P, 2 * R, W]),
        op=ALU.add,
    )
    nc.scalar.activation(th[:], e[:], AF.Tanh)
    PL = pq.tile([1, P], f32, tag="sm")
    for m in range(2):
        nc.tensor.matmul(
            PL[:], lhsT=sm[:, C_WPROJ + m : C_WPROJ + m + 1],
            rhs=th[:, m * P : (m + 1) * P],
            start=(m == 0), stop=(m == 1),
        )

    # ---------------- banded softmax ------------------------------------
    rmax = cp.tile([1, R], f32)
    nc.vector.reduce_max(
        out=rmax[:], in_=PL[:].rearrange("p (r j) -> p r j", j=W),
        axis=mybir.AxisListType.X,
    )
    tsub = cp.tile([1, P], f32)
    nc.vector.tensor_tensor(
        out=tsub[:].rearrange("p (r j) -> p r j", j=W),
        in0=PL[:].rearrange("p (r j) -> p r j", j=W),
        in1=rmax[:].to_broadcast([1, R, W]),
        op=ALU.subtract,
    )
    pex = cp.tile([1, P], f32)
    nc.scalar.activation(pex[:], tsub[:], AF.Exp)

    ji = cp.tile([1, P], i32)
    nc.gpsimd.iota(
        ji[:].rearrange("p (r j) -> p r j", j=W),
        pattern=[[0, R], [1, W]],
        base=0,
        channel_multiplier=0,
    )
    pos = cp.tile([1, P], f32)
    nc.vector.tensor_copy(pos[:], ji[:])
    nc.vector.tensor_tensor(out=pos[:], in0=pos[:], in1=sB[:], op=ALU.add)
    m1 = cp.tile([1, P], f32)
    nc.vector.tensor_tensor(
        out=m1[:].rearrange("p (r j) -> p r j", j=W),
        in0=pos[:].rearrange("p (r j) -> p r j", j=W),
        in1=lo[:].to_broadcast([1, R, W]),
        op=ALU.is_ge,
    )
    m2t = cp.tile([1, P], f32)
    nc.vector.tensor_tensor(
        out=m2t[:].rearrange("p (r j) -> p r j", j=W),
        in0=pos[:].rearrange("p (r j) -> p r j", j=W),
        in1=hi[:].to_broadcast([1, R, W]),
        op=ALU.is_le,
    )
    nc.vector.tensor_tensor(out=m1[:], in0=m1[:], in1=m2t[:], op=ALU.mult)
    num = cp.tile([1, P], f32)
    nc.vector.tensor_tensor(out=num[:], in0=pex[:], in1=m1[:], op=ALU.mult)
    den = cp.tile([1, R], f32)
    nc.vector.reduce_sum(
        out=den[:], in_=num[:].rearrange("p (r j) -> p r j", j=W),
        axis=mybir.AxisListType.X,
    )
    nc.vector.tensor_scalar_max(den[:], den[:], 1e-12)
    rden = cp.tile([1, R], f32)
    nc.vector.reciprocal(rden[:], den[:])
    vals = cp.tile([1, P], f32)
    nc.vector.tensor_tensor(
        out=vals[:].rearrange("p (r j) -> p r j", j=W),
        in0=num[:].rearrange("p (r j) -> p r j", j=W),
        in1=rden[:].to_broadcast([1, R, W]),
        op=ALU.mult,
    )

    # ---------------- scatter band values into zeroed output -------------
    engs = [nc.scalar, nc.sync, nc.gpsimd, nc.scalar]
    for r in range(R):
        d = engs[r].dma_start(
            out=x["out"][r : r + 1, bass.ds(svals[r], W)],
            in_=vals[0:1, r * W : (r + 1) * W],
        )
        tile.add_dep_helper(d.ins, zero_dma.ins, reason="scatter after zero")



def _split_sync_waits(nc, cap: int = 1):
    """This walrus build rejects instructions carrying several semaphore
    waits.  Engines execute their stream in order, so hoisting excess waits
    onto wait-carriers inserted immediately before the instruction is
    equivalent."""
    f = nc.m.functions[0]
    uid = [0]
    for blk in f.blocks:
        insts = blk.instructions
        out = []
        for inst in insts:
            si = inst.sync_info
            waits = list(si.on_wait) if (si is not None and si.on_wait) else []
            if len(waits) > cap:
                keep, excess = waits[:cap], waits[cap:]
                for k in range(0, len(excess), cap):
                    nop = mybir.InstEventSemaphore(
                        name=f"{inst.name}-ws{uid[0]}",
                        engine=inst.engine,
                        ins=[],
                        outs=[],
                        sync_info=mybir.SyncInfo(
                            on_wait=excess[k : k + cap], on_update=[]
                        ),
                    )
                    uid[0] += 1
                    out.append(nop)
                inst.sync_info = mybir.SyncInfo(
                    on_wait=keep, on_update=list(si.on_update or [])
                )
            out.append(inst)
        blk.instructions = out


def build_graph(reps: int = 1, split_waits: bool = True):
    from contextlib import ExitStack

    nc = bass.Bass()
    x = {}
    x["pa"] = nc.declare_dram_parameter("pa", [R, T], f32, isOutput=False)
    x["pa_pad"] = nc.declare_dram_parameter("pa_pad", [R, TP], f32, isOutput=False)
    x["enc"] = nc.declare_dram_parameter("enc", [R * T, ENC_H], bf16, isOutput=False)
    x["smimg"] = nc.declare_dram_parameter("smimg", [P, SMC], f32, isOutput=False)
    x["smimgb"] = nc.declare_dram_parameter("smimgb", [P, SMB], bf16, isOutput=False)
    x["wenc"] = nc.declare_dram_parameter("wenc", [P, 4 * ATT_H], bf16, isOutput=False)
    x["wp1"] = nc.declare_dram_parameter("wp1", [P, 2 * DEC_H], bf16, isOutput=False)
    x["wp1b"] = nc.declare_dram_parameter("wp1b", [16, 2 * DEC_H], bf16, isOutput=False)
    x["wp2"] = nc.declare_dram_parameter("wp2", [P, 8 * DEC_H], bf16, isOutput=False)
    x["wdec"] = nc.declare_dram_parameter("wdec", [P, 4 * ATT_H], bf16, isOutput=False)
    x["out"] = nc.declare_dram_parameter("out", [R, T], f32, isOutput=True)

    with tile.TileContext(nc) as tc:
        for _ in range(reps):
            with ExitStack() as ctx:
                _emit_core(nc, tc, ctx, x)
    if split_waits:
        _split_sync_waits(nc)
    return nc


def _fold(w, q, p=P):
    """[q*p, n] -> [p, q*n] with column-blocks per q (SBUF matmul layout)."""
    n = w.shape[1]
    return np.ascontiguousarray(
        w.reshape(q, p, n).transpose(1, 0, 2).reshape(p, q * n), dtype=np.float32
    )


def host_prep(inputs: dict) -> list:
    """Shard + lay out the full inputs into 8 per-core input maps."""
    inp = {k: np.asarray(v) for k, v in inputs.items()}
    pa = inp["prev_attention"].astype(np.float32)[:, :, 0]        # [N, T]
    enc = inp["input_enc"].astype(np.float32)                      # [N, T, E]
    dec = inp["input_dec"].astype(np.float32)[:, 0, :]             # [N, 80]
    spk = inp["spkr_vec"].astype(np.float32)[:, 0, :]              # [N, 64]
    spd = inp["speed"].astype(np.float32)                          # [N]
    lens = inp["lengths_enc"].astype(np.float32)                   # [N]

    import ml_dtypes

    bft = ml_dtypes.bfloat16
    wenc = _fold(np.asarray(inp["W_enc"], np.float32), 4).astype(bft)
    wp2 = _fold(np.asarray(inp["Wp2"], np.float32), 8).astype(bft)
    wdec = _fold(np.asarray(inp["W_dec"], np.float32), 4).astype(bft)
    wp1_full = np.asarray(inp["Wp1"], np.float32)
    wp1 = np.ascontiguousarray(wp1_full[0:128, :]).astype(bft)
    wp1b = np.ascontiguousarray(wp1_full[128:144, :]).astype(bft)

    smimg_base = np.zeros((P, SMC), np.float32)
    smimg_base[:, C_BP1 : C_BP1 + 8] = np.asarray(inp["bp1"], np.float32).reshape(8, P).T
    smimg_base[:, C_BP2 : C_BP2 + 4] = np.asarray(inp["bp2"], np.float32).reshape(4, P).T
    smimg_base[:, C_BENC : C_BENC + 2] = (
        np.asarray(inp["b_enc"], np.float32).reshape(2, P).T
    )
    smimg_base[:, C_WPROJ : C_WPROJ + 2] = (
        np.asarray(inp["W_proj"], np.float32).reshape(2, P).T
    )
    smimg_base[0:KW, C_CW : C_CW + ATT_H] = np.asarray(
        inp["conv_w"], np.float32
    )[:, 0, :].T
    smb_base = np.zeros((P, SMB), bft)
    smb_base[0:1, B_BENC : B_BENC + 2 * P] = (
        np.asarray(inp["b_enc"], np.float32).reshape(1, 2 * P).astype(bft)
    )
    smb_base[0:SPK, B_WSPK : B_WSPK + ATT_H] = np.asarray(
        inp["W_spkr"], np.float32
    ).astype(bft)
    smb_base[0:1, B_WSPD : B_WSPD + ATT_H] = (
        np.asarray(inp["W_speed"], np.float32).reshape(1, ATT_H).astype(bft)
    )

    in_maps = []
    for cix in range(NCORES):
        rows = slice(cix * R, (cix + 1) * R)
        pa_pad = np.zeros((R, TP), np.float32)
        pa_pad[:, PAD : PAD + T] = pa[rows]
        smimg = smimg_base.copy()
        smimg[0:1, C_LEN : C_LEN + R] = lens[rows].reshape(1, R)
        smb = smb_base.copy()
        ds_t = np.concatenate([dec[rows], spk[rows]], axis=1).T  # [144, R]
        smb[:, B_DST0 : B_DST0 + R] = ds_t[0:128, :].astype(bft)
        smb[0:16, B_DST1 : B_DST1 + R] = ds_t[128:144, :].astype(bft)
        smb[0:SPK, B_SPK : B_SPK + R] = spk[rows].T.astype(bft)
        smb[0:1, B_SPD : B_SPD + R] = spd[rows].reshape(1, R).astype(bft)
        m = {
            "pa": np.ascontiguousarray(pa[rows]),
            "pa_pad": pa_pad,
            "enc": np.ascontiguousarray(enc[rows].reshape(R * T, ENC_H)).astype(bft),
            "smimg": smimg,
            "smimgb": smb,
            "wenc": wenc,
            "wp1": wp1,
            "wp1b": wp1b,
            "wp2": wp2,
            "wdec": wdec,
        }
        in_maps.append(m)
    return in_maps


_CACHED = {}


def kernel(**inputs) -> np.ndarray:
    from concourse.bass_utils import run_bass_kernel_spmd

    if "nc" not in _CACHED:
        _CACHED["nc"] = build_graph()
    nc = _CACHED["nc"]
    in_maps = host_prep(inputs)
    res = run_bass_kernel_spmd(nc, in_maps, core_ids=list(range(NCORES)))
    out = np.empty((N, T, 1), np.float32)
    for cix in range(NCORES):
        out[cix * R : (cix + 1) * R, :, 0] = res.results[cix]["out"]
    return out
